# revision 13
# baseline (speedup 1.0000x reference)
"""Bayes classifier logits on 8 Trainium2 NeuronCores.

logits[b, c] = log w_c - 0.5 * (maha_cb + logdet_c + D*log(2pi))
maha_cb = (x_b - mu_c)^T P_c (x_b - mu_c),  P_c = covs_c^{-1}

Data-parallel over batch (8 cores).  Per core:
  logits[b, c] = const_c + q_c . x_b - 0.5 x_b^T P_c x_b
The quadratic term is one long PSUM-accumulated matmul over "squared-sum"
features, using x_i x_j = ((x_i+x_j)^2 - x_i^2 - x_j^2)/2 folded into
host-precomputed weights:
  S   = E @ X^T        (PE; E rows are e_i / e_i+e_j 0-1 patterns, 2080 rows)
  Phi = S^2            (ACT/DVE square during PSUM->SBUF evacuation, bf16)
  acc = sum_k Wq_k^T Phi_k  + Waug^T [X^T; 1]   (PE, PSUM accumulate, fp32)
"""

import numpy as np
import ml_dtypes

import concourse.bass as bass
from concourse import bacc, mybir, tile
from concourse.bass_utils import run_bass_kernel_spmd

B, C, D = 32768, 100, 64
N_CORES = 8
BS = B // N_CORES          # 4096 samples per core
NP_ = 512                  # samples per pass (one PSUM bank, fp32)
N_PASS = BS // NP_         # 8
N_PAIR = D * (D - 1) // 2  # 2016
N_FEAT = D + N_PAIR        # 2080 (singles first, then pairs i<j)
N_STORE = 18               # stored K-chunk slots (pad so chunks pair up 2x2)
N_CHUNK = 17               # K-chunks actually computed (2080 rows + 96 pad)
FEAT_PAD = N_STORE * 128   # 2304
N_GRP = N_STORE // 2       # 9 row-tiled chunk pairs
N_DVE_SQ = 1               # of every 3 groups, this many go to DVE (2-step)
EPI_ON_ACT = False         # acc->SBUF epilogue engine (DVE balances ACT)

_BF16 = mybir.dt.bfloat16
_F32 = mybir.dt.float32


def _host_prep(x, means, covs, weights):
    """Numpy (fp64) precompute of device weight operands."""
    mu = np.asarray(means).astype(np.float64)
    cv = np.asarray(covs).astype(np.float64)
    w = np.asarray(weights).astype(np.float64)

    L = np.linalg.cholesky(cv)                       # [C, D, D]
    logdet = 2.0 * np.sum(np.log(np.diagonal(L, axis1=1, axis2=2)), axis=1)
    P = np.linalg.inv(cv)                            # [C, D, D] (SPD)
    P = 0.5 * (P + np.transpose(P, (0, 2, 1)))
    q = np.einsum("cij,cj->ci", P, mu)               # [C, D]
    const = (np.log(w) - 0.5 * (logdet + D * np.log(2.0 * np.pi)
                                + np.einsum("ci,ci->c", mu, q)))

    iu, ju = np.triu_indices(D, k=1)                 # pair order (i<j)

    # E: [FEAT_PAD, D] 0/1 sum patterns.
    E = np.zeros((FEAT_PAD, D), dtype=np.float64)
    E[np.arange(D), np.arange(D)] = 1.0
    E[D + np.arange(N_PAIR), iu] = 1.0
    E[D + np.arange(N_PAIR), ju] = 1.0

    # Quadratic weights so that  sum_f Wq[f, c] * (E@x)_f^2 = -0.5 x^T P_c x
    Wq = np.zeros((FEAT_PAD, C), dtype=np.float64)
    Pij = P[:, iu, ju]                               # [C, N_PAIR]
    Wq[D + np.arange(N_PAIR), :] = (-0.5 * Pij).T
    Pdiag = np.diagonal(P, axis1=1, axis2=2)         # [C, D]
    offdiag_rowsum = P.sum(axis=2) - Pdiag
    Wq[np.arange(D), :] = (-0.5 * Pdiag + 0.5 * offdiag_rowsum).T

    # Sum-gen stationary operands: lhsT_k = E[128k:128(k+1), :].T -> [64, 128]
    # stacked in pairs so chunk 2g+1 lives at SBUF partitions 64..127:
    # et_store[[0:64], g, :]  = lhsT_{2g},  et_store[[64:128], g, :] = lhsT_{2g+1}
    lhsT = E.reshape(N_STORE, 128, D).transpose(0, 2, 1)   # [18, 64, 128]
    et_store = np.concatenate(
        [lhsT[0::2], lhsT[1::2]], axis=1).transpose(1, 0, 2)  # [128, 9, 128]

    # Main-matmul stationary: wq_store[:, k, :] = Wq[128k:128(k+1), :]
    wq_store = Wq.reshape(N_STORE, 128, C).transpose(1, 0, 2)  # [128, 18, C]

    # Aug (linear + const) fp32 weights: [D+1, C]
    waug = np.concatenate([q.T, const[None, :]], axis=0)

    return {
        "et": np.ascontiguousarray(et_store).astype(ml_dtypes.bfloat16),
        "wq": np.ascontiguousarray(wq_store).astype(ml_dtypes.bfloat16),
        "waug": np.ascontiguousarray(waug).astype(np.float32),
    }


def _build_program(repeat=1):
    nc = bacc.Bacc("TRN2", target_bir_lowering=False, debug=False,
                   num_devices=N_CORES)
    xstack_d = nc.dram_tensor("xstack", [128, BS], _BF16,
                              kind="ExternalInput").ap()     # [X^T; X^T] bf16
    xaug_d = nc.dram_tensor("xaug", [D + 1, BS], _F32,
                            kind="ExternalInput").ap()       # [X^T; ones] fp32
    et_d = nc.dram_tensor("et", [128, N_GRP, 128], _BF16,
                          kind="ExternalInput").ap()
    wq_d = nc.dram_tensor("wq", [128, N_STORE, C], _BF16,
                          kind="ExternalInput").ap()
    waug_d = nc.dram_tensor("waug", [D + 1, C], _F32,
                            kind="ExternalInput").ap()
    out_d = nc.dram_tensor("logits_t", [C, BS], _F32,
                           kind="ExternalOutput").ap()

    with tile.TileContext(nc) as tc:  # noqa: PLR1702
        with (
            tc.tile_pool(name="const", bufs=1) as cpool,
            tc.tile_pool(name="xin", bufs=3) as xpool,
            tc.tile_pool(name="phi", bufs=3) as phipool,
            tc.tile_pool(name="outp", bufs=2) as opool,
            tc.tile_pool(name="psum_s", bufs=2, space="PSUM") as spsum,
            tc.tile_pool(name="psum_o", bufs=2, space="PSUM") as opsum,
        ):
            et_t = cpool.tile([128, N_GRP, 128], _BF16)
            nc.sync.dma_start(et_t[:], et_d[:])
            wq_t = cpool.tile([128, N_STORE, C], _BF16)
            nc.sync.dma_start(wq_t[:], wq_d[:])
            waug_t = cpool.tile([D + 1, C], _F32)
            nc.sync.dma_start(waug_t[:], waug_d[:])

            for _rep in range(repeat):
              for p in range(N_PASS):
                ns = bass.ts(p, NP_)
                xs = xpool.tile([128, NP_], _BF16, tag="xs")
                nc.sync.dma_start(xs[:], xstack_d[:, ns])
                xa = xpool.tile([D + 1, NP_], _F32, tag="xa")
                nc.sync.dma_start(xa[:], xaug_d[:, ns])

                # sum-gen (2x2 row-tiled PE); each chunk-pair's S lands in
                # one 2-bank PSUM tile so the square-evacuation is a single
                # fused op (ACT Square, or DVE copy+square for some groups).
                # Per-group phi tiles keep deps fine-grained so the main
                # accumulation starts as soon as group 0 is evacuated.
                phis = []
                for g in range(N_GRP):
                    k0 = 2 * g
                    if k0 >= N_CHUNK:
                        continue
                    dual = (k0 + 1) < N_CHUNK
                    nb = 2 if dual else 1
                    phig = phipool.tile([128, 2, NP_], _BF16, tag=f"phi{g}")
                    phis.append(phig)
                    s2 = spsum.tile([128, 2, NP_], _F32, tag="s")
                    nc.tensor.matmul(s2[:, 0, :], et_t[0:64, g, :],
                                     xs[0:64, :])
                    if dual:
                        nc.tensor.matmul(s2[:, 1, :], et_t[64:128, g, :],
                                         xs[64:128, :])
                    src = s2[:, 0:nb, :]
                    dst = phig[:, 0:nb, :]
                    if (g % 3) < N_DVE_SQ:
                        tmp = xpool.tile([128, 2, NP_], _BF16, tag="sqtmp")
                        nc.vector.tensor_copy(tmp[:, 0:nb, :], src)
                        nc.vector.tensor_tensor(
                            dst, tmp[:, 0:nb, :], tmp[:, 0:nb, :],
                            mybir.AluOpType.mult)
                    else:
                        nc.scalar.activation(
                            dst, src, mybir.ActivationFunctionType.Square)

                # main accumulation matmul
                acc = opsum.tile([C, NP_], _F32, tag="acc")
                for k in range(N_CHUNK):
                    nc.tensor.matmul(
                        acc[:], wq_t[:, k, :], phis[k // 2][:, k % 2, :],
                        start=(k == 0), stop=False)
                nc.tensor.matmul(acc[:], waug_t[:], xa[:],
                                 start=False, stop=True)

                ot = opool.tile([C, NP_], _F32, tag="ot")
                if EPI_ON_ACT:
                    nc.scalar.copy(ot[:], acc[:])
                else:
                    nc.vector.tensor_copy(ot[:], acc[:])
                nc.sync.dma_start(out_d[:, ns], ot[:])

    nc.compile()
    return nc


_NC_CACHE = None


def _get_nc():
    global _NC_CACHE
    if _NC_CACHE is None:
        _NC_CACHE = _build_program()
    return _NC_CACHE


def _make_in_maps(x, prep):
    x = np.asarray(x)
    in_maps = []
    for c in range(N_CORES):
        xs = x[c * BS:(c + 1) * BS].astype(np.float32)     # [BS, D]
        xt = np.ascontiguousarray(xs.T)                    # [D, BS]
        xstack = np.concatenate([xt, xt], axis=0)
        xaug = np.concatenate([xt, np.ones((1, BS), np.float32)], axis=0)
        in_maps.append({
            "xstack": np.ascontiguousarray(xstack.astype(ml_dtypes.bfloat16)),
            "xaug": np.ascontiguousarray(xaug.astype(np.float32)),
            "et": prep["et"],
            "wq": prep["wq"],
            "waug": prep["waug"],
        })
    return in_maps


def kernel(x, means, covs, weights):
    x = np.asarray(x)
    prep = _host_prep(x, means, covs, weights)
    nc = _get_nc()
    res = run_bass_kernel_spmd(nc, _make_in_maps(x, prep),
                               list(range(N_CORES)))
    outs = [res.results[c]["logits_t"] for c in range(N_CORES)]  # [C, BS]
    logits_t = np.concatenate(outs, axis=1)                      # [C, B]
    return np.ascontiguousarray(logits_t.T.astype(np.float32))   # [B, C]


# revision 16
# speedup vs baseline: 6.1869x; 6.1869x over previous
"""Bayes classifier logits on 8 Trainium2 NeuronCores.

logits[b, c] = log w_c - 0.5 * (maha_cb + logdet_c + D*log(2pi))
maha_cb = (x_b - mu_c)^T P_c (x_b - mu_c),  P_c = covs_c^{-1}

Data-parallel over batch (8 cores).  Per core:
  logits[b, c] = const_c + q_c . x_b - 0.5 x_b^T P_c x_b
The quadratic term is one long PSUM-accumulated matmul over "squared-sum"
features, using x_i x_j = ((x_i+x_j)^2 - x_i^2 - x_j^2)/2 folded into
host-precomputed weights:
  S   = E @ X^T        (PE; E rows are e_i / e_i+e_j 0-1 patterns, 2080 rows)
  Phi = S^2            (ACT/DVE square during PSUM->SBUF evacuation, bf16)
  acc = sum_k Wq_k^T Phi_k  + Waug^T [X^T; 1]   (PE, PSUM accumulate, fp32)
"""

import numpy as np
import ml_dtypes

import concourse.bass as bass
from concourse import bacc, mybir, tile
from concourse.bass_utils import run_bass_kernel_spmd

B, C, D = 32768, 100, 64
N_CORES = 8
BS = B // N_CORES          # 4096 samples per core
NP_ = 512                  # samples per pass (one PSUM bank, fp32)
N_PASS = BS // NP_         # 8
N_PAIR = D * (D - 1) // 2  # 2016
N_FEAT = D + N_PAIR        # 2080 (singles first, then pairs i<j)
N_STORE = 18               # stored K-chunk slots (pad so chunks pair up 2x2)
N_CHUNK = 17               # K-chunks actually computed (2080 rows + 96 pad)
FEAT_PAD = N_STORE * 128   # 2304
N_GRP = N_STORE // 2       # 9 row-tiled chunk pairs
N_DVE_SQ = 1               # of every 3 groups, this many go to DVE (2-step)
EPI_ON_ACT = True          # acc->SBUF epilogue engine
PHI_BUFS = 2               # phi pool buffers

_BF16 = mybir.dt.bfloat16
_F32 = mybir.dt.float32


def _host_prep(x, means, covs, weights):
    """Numpy (fp64) precompute of device weight operands."""
    mu = np.asarray(means).astype(np.float64)
    cv = np.asarray(covs).astype(np.float64)
    w = np.asarray(weights).astype(np.float64)

    L = np.linalg.cholesky(cv)                       # [C, D, D]
    logdet = 2.0 * np.sum(np.log(np.diagonal(L, axis1=1, axis2=2)), axis=1)
    P = np.linalg.inv(cv)                            # [C, D, D] (SPD)
    P = 0.5 * (P + np.transpose(P, (0, 2, 1)))
    q = np.einsum("cij,cj->ci", P, mu)               # [C, D]
    const = (np.log(w) - 0.5 * (logdet + D * np.log(2.0 * np.pi)
                                + np.einsum("ci,ci->c", mu, q)))

    iu, ju = np.triu_indices(D, k=1)                 # pair order (i<j)

    # E: [FEAT_PAD, D] 0/1 sum patterns.
    E = np.zeros((FEAT_PAD, D), dtype=np.float64)
    E[np.arange(D), np.arange(D)] = 1.0
    E[D + np.arange(N_PAIR), iu] = 1.0
    E[D + np.arange(N_PAIR), ju] = 1.0

    # Quadratic weights so that  sum_f Wq[f, c] * (E@x)_f^2 = -0.5 x^T P_c x
    Wq = np.zeros((FEAT_PAD, C), dtype=np.float64)
    Pij = P[:, iu, ju]                               # [C, N_PAIR]
    Wq[D + np.arange(N_PAIR), :] = (-0.5 * Pij).T
    Pdiag = np.diagonal(P, axis1=1, axis2=2)         # [C, D]
    offdiag_rowsum = P.sum(axis=2) - Pdiag
    Wq[np.arange(D), :] = (-0.5 * Pdiag + 0.5 * offdiag_rowsum).T

    # Linear + const terms folded into chunk 16's padding rows (the device
    # fills the matching phi rows with [ones; zeros; X^T]).  const is split
    # hi/lo across two ones-rows to survive the bf16 weight cast.
    import ml_dtypes as _md
    c_hi = np.asarray(const.astype(_md.bfloat16), dtype=np.float64)
    Wq[N_FEAT, :] = c_hi                             # row 2080: ones * hi
    Wq[N_FEAT + 1, :] = const - c_hi                 # row 2081: ones * lo
    Wq[N_FEAT + 32:N_FEAT + 96, :] = q.T             # rows 2112..2175: x_i

    # Sum-gen stationary operands: lhsT_k = E[128k:128(k+1), :].T -> [64, 128]
    # stacked in pairs so chunk 2g+1 lives at SBUF partitions 64..127:
    # et_store[[0:64], g, :]  = lhsT_{2g},  et_store[[64:128], g, :] = lhsT_{2g+1}
    lhsT = E.reshape(N_STORE, 128, D).transpose(0, 2, 1)   # [18, 64, 128]
    et_store = np.concatenate(
        [lhsT[0::2], lhsT[1::2]], axis=1).transpose(1, 0, 2)  # [128, 9, 128]

    # Main-matmul stationary: wq_store[:, k, :] = Wq[128k:128(k+1), :]
    wq_store = Wq.reshape(N_STORE, 128, C).transpose(1, 0, 2)  # [128, 18, C]

    return {
        "et": np.ascontiguousarray(et_store).astype(ml_dtypes.bfloat16),
        "wq": np.ascontiguousarray(wq_store).astype(ml_dtypes.bfloat16),
    }


def _build_program(repeat=1):
    nc = bacc.Bacc("TRN2", target_bir_lowering=False, debug=False,
                   num_devices=N_CORES)
    xstack_d = nc.dram_tensor("xstack", [128, BS], _BF16,
                              kind="ExternalInput").ap()     # [X^T; X^T] bf16
    et_d = nc.dram_tensor("et", [128, N_GRP, 128], _BF16,
                          kind="ExternalInput").ap()
    wq_d = nc.dram_tensor("wq", [128, N_STORE, C], _BF16,
                          kind="ExternalInput").ap()
    out_d = nc.dram_tensor("logits_t", [C, BS], _F32,
                           kind="ExternalOutput").ap()

    with tile.TileContext(nc) as tc:  # noqa: PLR1702
        with (
            tc.tile_pool(name="const", bufs=1) as cpool,
            tc.tile_pool(name="xin", bufs=3) as xpool,
            tc.tile_pool(name="phi", bufs=PHI_BUFS) as phipool,
            tc.tile_pool(name="outp", bufs=2) as opool,
            tc.tile_pool(name="psum_s", bufs=2, space="PSUM") as spsum,
            tc.tile_pool(name="psum_o", bufs=2, space="PSUM") as opsum,
        ):
            et_t = cpool.tile([128, N_GRP, 128], _BF16)
            nc.sync.dma_start(et_t[:], et_d[:])
            wq_t = cpool.tile([128, N_STORE, C], _BF16)
            nc.sync.dma_start(wq_t[:], wq_d[:])

            for _rep in range(repeat):
              for p in range(N_PASS):
                ns = bass.ts(p, NP_)
                xs = xpool.tile([128, NP_], _BF16, tag="xs")
                nc.sync.dma_start(xs[:], xstack_d[:, ns])

                # sum-gen (2x2 row-tiled PE); each chunk-pair's S lands in
                # one 2-bank PSUM tile so the square-evacuation is a single
                # fused op (ACT Square, or DVE copy+square for some groups).
                # Per-group phi tiles keep deps fine-grained so the main
                # accumulation starts as soon as group 0 is evacuated.
                phis = []
                for g in range(N_GRP):
                    k0 = 2 * g
                    if k0 >= N_CHUNK:
                        continue
                    dual = (k0 + 1) < N_CHUNK
                    nb = 2 if dual else 1
                    phig = phipool.tile([128, 2, NP_], _BF16, tag=f"phi{g}")
                    phis.append(phig)
                    s2 = spsum.tile([128, 2, NP_], _F32, tag="s")
                    if dual:
                        nc.tensor.matmul(s2[:, 0, :], et_t[0:64, g, :],
                                         xs[0:64, :])
                        nc.tensor.matmul(s2[:, 1, :], et_t[64:128, g, :],
                                         xs[64:128, :])
                        src = s2[:, 0:nb, :]
                        dst = phig[:, 0:nb, :]
                    else:
                        # solo chunk 16: rows 0-31 squares; rows 32/33 ones
                        # (const hi/lo); 34-63 zero; 64-127 = X^T (linear).
                        nc.tensor.matmul(s2[0:32, 0, :],
                                         et_t[0:64, g, 0:32], xs[0:64, :])
                        nc.gpsimd.memset(phig[32:64, 0, :], 0.0)
                        nc.gpsimd.memset(phig[32:34, 0, :], 1.0)
                        nc.sync.dma_start(phig[64:128, 0, :], xs[0:64, :])
                        src = s2[0:32, 0:1, :]
                        dst = phig[0:32, 0:1, :]
                    if (g % 3) < N_DVE_SQ:
                        tmp = xpool.tile([128, 2, NP_], _BF16, tag="sqtmp")
                        nc.vector.tensor_copy(tmp[:, 0:nb, :], src)
                        nc.vector.tensor_tensor(
                            dst, tmp[:, 0:nb, :], tmp[:, 0:nb, :],
                            mybir.AluOpType.mult)
                    else:
                        nc.scalar.activation(
                            dst, src, mybir.ActivationFunctionType.Square)

                # main accumulation matmul
                acc = opsum.tile([C, NP_], _F32, tag="acc")
                for k in range(N_CHUNK):
                    nc.tensor.matmul(
                        acc[:], wq_t[:, k, :], phis[k // 2][:, k % 2, :],
                        start=(k == 0), stop=(k == N_CHUNK - 1))

                ot = opool.tile([C, NP_], _F32, tag="ot")
                if EPI_ON_ACT:
                    nc.scalar.copy(ot[:], acc[:])
                else:
                    nc.vector.tensor_copy(ot[:], acc[:])
                nc.sync.dma_start(out_d[:, ns], ot[:])

    nc.compile()
    return nc


_NC_CACHE = None


def _get_nc():
    global _NC_CACHE
    if _NC_CACHE is None:
        _NC_CACHE = _build_program()
    return _NC_CACHE


def _make_in_maps(x, prep):
    x = np.asarray(x)
    in_maps = []
    for c in range(N_CORES):
        xs = x[c * BS:(c + 1) * BS].astype(np.float32)     # [BS, D]
        xt = np.ascontiguousarray(xs.T)                    # [D, BS]
        xstack = np.concatenate([xt, xt], axis=0)
        in_maps.append({
            "xstack": np.ascontiguousarray(xstack.astype(ml_dtypes.bfloat16)),
            "et": prep["et"],
            "wq": prep["wq"],
        })
    return in_maps


def kernel(x, means, covs, weights):
    x = np.asarray(x)
    prep = _host_prep(x, means, covs, weights)
    nc = _get_nc()
    res = run_bass_kernel_spmd(nc, _make_in_maps(x, prep),
                               list(range(N_CORES)))
    outs = [res.results[c]["logits_t"] for c in range(N_CORES)]  # [C, BS]
    logits_t = np.concatenate(outs, axis=1)                      # [C, B]
    return np.ascontiguousarray(logits_t.T.astype(np.float32))   # [B, C]
